# revision 3
# baseline (speedup 1.0000x reference)
"""Trainium2 Bass kernel v2 for CausalSelfAttentionLayer (B=4 L=1024 C=512).

Sharding over 8 NeuronCores: core c -> sequence b=c//2, head-group g=c%2
(8 of 16 heads).  All GEMMs run in bf16 (stationary+moving) with f32 PSUM
accumulate.  Per core:
  A: LN (bn_stats DVE, apply on ACT -> bf16), x-transpose via DMA XBAR
     (dma_start_transpose), weights prefetched per head-pair (double buffer).
  B: per head-pair hp: QKV matmuls (bias folded into the PSUM->SBUF copy on
     ACT/DVE), attention with transposed probabilities, causal boundary
     narrowing (masked-out column ranges skipped), exp on ACT -> bf16,
     row-sums via ones-matmul accumulated in PSUM, normalize on DVE.
  C: c_proj partials (bf16) in token-pair order (j, j+4) with 4 chunked
     pair-ReduceScatters so all but the last overlap compute; out write per
     chunk (bf16->f32 DMA convert).
"""
import sys

for _p in ("/opt/trn_rl_repo", "/root/.axon_site/_ro/trn_rl_repo"):
    if _p not in sys.path:
        sys.path.append(_p)

from contextlib import ExitStack

import numpy as np

import concourse.bass as bass
import concourse.mybir as mybir
import concourse.tile as tile
from concourse import bacc
from concourse.bass_utils import run_bass_kernel_spmd

B, L, C, H, D = 4, 1024, 512, 2048, 128
NHL = 8          # heads per core
NPAIR = 4        # head pairs per core
T = L            # tokens per core (one sequence)
EPS = 1e-5
f32 = mybir.dt.float32
bf16 = mybir.dt.bfloat16

_CACHE = {}


def _build(collective=True, nbody=1, nrs=1):
    nc = bacc.Bacc("TRN2", target_bir_lowering=False, debug=False, num_devices=8)

    x_t = nc.dram_tensor("x", [T, H], f32, kind="ExternalInput")
    # per head-pair stationary weights [128, 16 cgroups, 768 cols]
    # col order: q jh0, q jh1, k jh0, k jh1, v (256)
    w_t = nc.dram_tensor("wqkv", [NPAIR, 128, 16, 768], bf16, kind="ExternalInput")
    qkb_t = nc.dram_tensor("qkbias", [128, 16], f32, kind="ExternalInput")
    vb_t = nc.dram_tensor("vbias", [1, 1024], bf16, kind="ExternalInput")
    kc_t = nc.dram_tensor("kcT", [128, NHL, C], bf16, kind="ExternalInput")
    vc_t = nc.dram_tensor("vcp", [128, 4, NHL, D], bf16, kind="ExternalInput")
    pw_t = nc.dram_tensor("pw", [128, NHL, H], bf16, kind="ExternalInput")
    pb_t = nc.dram_tensor("pb", [1, H], bf16, kind="ExternalInput")
    mask_t = nc.dram_tensor("mask", [128, 512], bf16, kind="ExternalInput")
    id_t = nc.dram_tensor("ident", [128, 128], bf16, kind="ExternalInput")
    onc_t = nc.dram_tensor("onescol", [128, 1], bf16, kind="ExternalInput")
    out_t = nc.dram_tensor("out", [T // 2, H], f32, kind="ExternalOutput")

    Exp = mybir.ActivationFunctionType.Exp
    Sqrt = mybir.ActivationFunctionType.Sqrt
    Ident = mybir.ActivationFunctionType.Identity
    mult = mybir.AluOpType.mult

    with tile.TileContext(nc) as tc, ExitStack() as _stk:
        def _pool(name, bufs, **kw):
            return _stk.enter_context(tc.tile_pool(name=name, bufs=bufs, **kw))
        cst = _pool("cst", 1)
        xload = _pool("xload", 2)
        xlnp = _pool("xln", 2)
        st = _pool("st", 8)
        bigx = _pool("bigx", 2)
        wp = _pool("wp", 2)
        pwp = _pool("pwp", 1)
        qkvp = _pool("qkv", 6)
        kvc = _pool("kvc", 4)
        ptp = _pool("ptp", 4)
        rb = _pool("rb", 2)
        otp = _pool("otp", 1)
        po = _pool("po", 2)
        psS = _pool("psS", 8, space="PSUM")
        dram = _pool("dram", 1, space="DRAM")

        # ---- constants ----
        mask = cst.tile([128, 512], bf16)
        nc.gpsimd.dma_start(out=mask, in_=mask_t.ap())
        ident = cst.tile([128, 128], bf16)
        nc.gpsimd.dma_start(out=ident, in_=id_t.ap())
        onescol = cst.tile([128, 1], bf16)
        nc.gpsimd.dma_start(out=onescol, in_=onc_t.ap())
        qkbias = cst.tile([128, 16], f32)
        nc.gpsimd.dma_start(out=qkbias, in_=qkb_t.ap())
        vbias = cst.tile([128, 1024], bf16)
        nc.gpsimd.dma_start(
            out=vbias,
            in_=bass.AP(tensor=vb_t.ap().tensor, offset=0,
                        ap=[[0, 128], [1, 1024]]))
        pbb = cst.tile([128, H], bf16)
        nc.gpsimd.dma_start(
            out=pbb,
            in_=bass.AP(tensor=pb_t.ap().tensor, offset=0,
                        ap=[[0, 128], [1, H]]))
        epst = cst.tile([128, 1], f32)
        nc.vector.memset(epst, EPS)

        partial = dram.tile([4, 2, 128, H], bf16)
        rsout = dram.tile([4, 128, H], bf16)

        for _rep in range(nbody):
            xTa = bigx.tile([128, 16, T // 2], bf16, tag="xT", name="xTa")
            xTb = bigx.tile([128, 16, T // 2], bf16, tag="xT", name="xTb")

            def ln_chunk(tch):
                """LN one 128-token chunk and DMA-transpose into xT."""
                xh = xload.tile([128, 2048], f32, tag="x", name="xh")
                nc.sync.dma_start(
                    out=xh[:, 0:1024],
                    in_=x_t.ap()[tch * 128:(tch + 1) * 128, 0:1024])
                nc.scalar.dma_start(
                    out=xh[:, 1024:2048],
                    in_=x_t.ap()[tch * 128:(tch + 1) * 128, 1024:2048])
                stats = st.tile([128, 4, 6], f32, tag="stats")
                for sg in range(4):
                    nc.vector.bn_stats(
                        out=stats[:, sg, :],
                        in_=xh[:, sg * 512:(sg + 1) * 512])
                mv = st.tile([128, 2], f32, tag="mv")
                nc.vector.bn_aggr(out=mv, in_=stats)
                stdt = st.tile([128, 1], f32, tag="std")
                nc.scalar.activation(stdt, mv[:, 1:2], Sqrt, bias=epst[:, 0:1])
                rstd = st.tile([128, 1], f32, tag="rstd")
                nc.vector.reciprocal(rstd, stdt)
                m2p = st.tile([128, 1], f32, tag="m2p")
                nc.vector.tensor_scalar(
                    out=m2p, in0=mv[:, 0:1],
                    scalar1=rstd[:, 0:1], scalar2=-1.0, op0=mult, op1=mult)
                xln = xlnp.tile([128, 2048], bf16, tag="xln", name="xln")
                nc.scalar.activation(
                    xln[:], xh[:], Ident,
                    bias=m2p[:, 0:1], scale=rstd[:, 0:1])
                xTh = xTa if tch < 4 else xTb
                tc4 = (tch % 4) * 128
                for g4 in range(4):
                    tp = psS.tile([128, 512], f32, tag="psS", name="tp")
                    for i4 in range(4):
                        nc.tensor.transpose(
                            tp[:, i4 * 64:(i4 + 1) * 64].bitcast(bf16),
                            xln[:, (g4 * 4 + i4) * 128:(g4 * 4 + i4 + 1) * 128],
                            ident)
                    nc.vector.tensor_copy(
                        out=xTh[:, g4 * 4:(g4 + 1) * 4, tc4:tc4 + 128],
                        in_=tp[:, 0:256].bitcast(bf16))

            def load_w(hp):
                w = wp.tile([128, 16, 768], bf16, tag="w", name=f"w{hp}")
                nc.scalar.dma_start(out=w, in_=w_t.ap()[hp])
                return w

            def load_cache(hp):
                kcT = kvc.tile([128, 2, C], bf16, tag="kv", name="kcT")
                nc.scalar.dma_start(
                    out=kcT, in_=kc_t.ap()[:, hp * 2:hp * 2 + 2, :])
                vcp = kvc.tile([128, 4, 2, 128], bf16, tag="kv", name="vcp")
                nc.scalar.dma_start(
                    out=vcp, in_=vc_t.ap()[:, :, hp * 2:hp * 2 + 2, :])
                return kcT, vcp

            def emit_qk(dst, w, hp, qk, jh, tt):
                """One [128 feat, 512 tok] slab of qT or kT."""
                xTh = xTa if tt == 0 else xTb
                pq = psS.tile([128, 512], f32, tag="psS", name="pq")
                for cg in range(16):
                    nc.tensor.matmul(
                        pq[:],
                        w[:, cg, qk * 256 + jh * 128:qk * 256 + (jh + 1) * 128],
                        xTh[:, cg, :],
                        start=(cg == 0), stop=(cg == 15))
                nc.scalar.activation(
                    dst[:, jh, tt * 512:(tt + 1) * 512], pq[:], Ident,
                    bias=qkbias[:, hp * 4 + qk * 2 + jh:hp * 4 + qk * 2 + jh + 1])

            def emit_v(vtok, w, hp, tch):
                xTh = xTa if tch < 4 else xTb
                pv = psS.tile([128, 512], f32, tag="psS", name="pv")
                for cg in range(16):
                    nc.tensor.matmul(
                        pv[:, 0:256],
                        xTh[:, cg, (tch % 4) * 128:(tch % 4 + 1) * 128],
                        w[:, cg, 512:768],
                        start=(cg == 0), stop=(cg == 15))
                nc.vector.tensor_add(
                    vtok[:, tch, :], pv[:, 0:256],
                    vbias[:, hp * 256:(hp + 1) * 256])

            def emit_attn(outT, qT, kT, vtok, kcT, vcp, hp, hl, qt):
                h = hp * 2 + hl
                # chunk list: (kind, idx, mask_start or None, col offset)
                chunks = [("c", pc, None, 0) for pc in range(4)]
                for kf in range(8):
                    dlt = qt * 512 - kf * 128
                    if dlt <= -512:
                        continue
                    ms = dlt + 384 if dlt < 127 else None
                    off = max(0, -dlt)
                    chunks.append(("f", kf, ms, off))

                po_ps = psS.tile([128, 512], f32, tag="psS", name="po_ps")
                sm_ps = psS.tile([128, 512], f32, tag="psS", name="sm_ps")

                pending = []

                def stage1(kind, idx, ms, off):
                    w = 512 - off
                    sc_ps = psS.tile([128, 512], f32, tag="psS", name="sc_ps")
                    lhs = (kcT[:, hl, idx * 128:(idx + 1) * 128] if kind == "c"
                           else kT[:, hl, idx * 128:(idx + 1) * 128])
                    nc.tensor.matmul(
                        sc_ps[:, 0:w], lhs,
                        qT[:, hl, qt * 512 + off:(qt + 1) * 512],
                        start=True, stop=True)
                    pt = ptp.tile([128, 512], bf16, tag="pt", name="pt")
                    nc.scalar.activation(pt[:, 0:w], sc_ps[:, 0:w], Exp)
                    if ms is not None:
                        nc.vector.tensor_mul(
                            pt[:, 0:w], pt[:, 0:w],
                            mask[:, 0:ms + 128])
                    return pt

                def stage2(pt, kind, idx, ms, off, first):
                    w = 512 - off
                    vchunk = (vcp[:, idx, hl, :] if kind == "c"
                              else vtok[:, idx, hl * 128:(hl + 1) * 128])
                    nc.tensor.matmul(po_ps[:, off:512], vchunk, pt[:, 0:w],
                                     start=first, stop=False)
                    nc.tensor.matmul(sm_ps[0:1, off:512], onescol, pt[:, 0:w],
                                     start=first, stop=False)

                first = True
                for ci, (kind, idx, ms, off) in enumerate(chunks):
                    pt = stage1(kind, idx, ms, off)
                    pending.append((pt, kind, idx, ms, off))
                    if ci >= 1:
                        args = pending.pop(0)
                        stage2(*args, first)
                        first = False
                for args in pending:
                    stage2(*args, first)
                    first = False

                rc = st.tile([1, 512], bf16, tag="recip", bufs=2)
                with nc.allow_low_precision(reason="softmax denom bf16"):
                    nc.vector.reciprocal(rc, sm_ps[0:1, :])
                rcb = rb.tile([128, 512], bf16, tag="rcb", name="rcb")
                nc.gpsimd.partition_broadcast(rcb[:], rc[:])
                nc.vector.tensor_mul(
                    out=outT[:, h, qt * 512:(qt + 1) * 512],
                    in0=po_ps[:], in1=rcb[:])

            # ---- phase A + hp0 QKV interleave ----
            w0 = load_w(0)
            kcT0, vcp0 = load_cache(0)
            outT = otp.tile([128, NHL, T], bf16, tag="outT")
            qT0 = qkvp.tile([128, 2, T], bf16, tag="qkv", name="qT0")
            kT0 = qkvp.tile([128, 2, T], bf16, tag="qkv", name="kT0")
            vtok0 = qkvp.tile([128, 8, 256], bf16, tag="qkv", name="vtok0")
            for tch in range(4):
                ln_chunk(tch)
            for jh in range(2):
                emit_qk(qT0, w0, 0, 0, jh, 0)
            for jh in range(2):
                emit_qk(kT0, w0, 0, 1, jh, 0)
            for tch in range(4):
                emit_v(vtok0, w0, 0, tch)
            for tch in range(4, 8):
                ln_chunk(tch)
            pw_sb = pwp.tile([128, NHL, H], bf16, tag="pw")
            nc.sync.dma_start(out=pw_sb, in_=pw_t.ap())
            for jh in range(2):
                emit_qk(qT0, w0, 0, 0, jh, 1)
            for jh in range(2):
                emit_qk(kT0, w0, 0, 1, jh, 1)
            for tch in range(4, 8):
                emit_v(vtok0, w0, 0, tch)

            # ---- phase B ----
            cur_w, cur_kc, cur_vc = w0, kcT0, vcp0
            qT, kT, vtok = qT0, kT0, vtok0
            for hp in range(NPAIR):
                if hp > 0:
                    qT = qkvp.tile([128, 2, T], bf16, tag="qkv", name="qT")
                    kT = qkvp.tile([128, 2, T], bf16, tag="qkv", name="kT")
                    vtok = qkvp.tile([128, 8, 256], bf16, tag="qkv",
                                     name="vtok")
                    for jh in range(2):
                        for tt in range(2):
                            emit_qk(qT, cur_w, hp, 0, jh, tt)
                    for jh in range(2):
                        for tt in range(2):
                            emit_qk(kT, cur_w, hp, 1, jh, tt)
                    for tch in range(8):
                        emit_v(vtok, cur_w, hp, tch)
                nxt = None
                if hp + 1 < NPAIR:
                    nxt = (load_w(hp + 1),) + load_cache(hp + 1)
                for qt in range(2):
                    for hl in range(2):
                        emit_attn(outT, qT, kT, vtok, cur_kc, cur_vc,
                                  hp, hl, qt)
                if nxt is not None:
                    cur_w, cur_kc, cur_vc = nxt

            # ---- phase C: c_proj partials + chunked pair-ReduceScatter ----
            for j in range(4):
                for gb in range(2):
                    tch = gb * 4 + j
                    for ht in range(4):
                        pp = psS.tile([128, 512], f32, tag="psS", name="pp")
                        for hh in range(NHL):
                            nc.tensor.matmul(
                                pp[:],
                                outT[:, hh, tch * 128:(tch + 1) * 128],
                                pw_sb[:, hh, ht * 512:(ht + 1) * 512],
                                start=(hh == 0), stop=(hh == NHL - 1))
                        ev = po.tile([128, 512], bf16, tag="po", name="ev")
                        nc.vector.tensor_add(ev[:], pp[:],
                                             pbb[:, ht * 512:(ht + 1) * 512])
                        nc.sync.dma_start(
                            out=partial[j, gb, :, ht * 512:(ht + 1) * 512],
                            in_=ev[:])
                if collective:
                    for _ in range(nrs):
                        nc.gpsimd.collective_compute(
                            "ReduceScatter",
                            mybir.AluOpType.add,
                            replica_groups=[[0, 1], [2, 3], [4, 5], [6, 7]],
                            ins=[partial[j]],
                            outs=[rsout[j]],
                        )
                    nc.gpsimd.dma_start(
                        out=out_t.ap()[j * 128:(j + 1) * 128, :],
                        in_=rsout[j])
                else:
                    nc.gpsimd.dma_start(
                        out=out_t.ap()[j * 128:(j + 1) * 128, :],
                        in_=partial[j, 0, :, :])

    nc.compile()
    return nc


def _host_prep(inputs):
    import ml_dtypes
    hidden = np.ascontiguousarray(np.asarray(inputs["hidden_states"],
                                             dtype=np.float32))
    k_cache = np.asarray(inputs["k_cache"], dtype=np.float32)
    v_cache = np.asarray(inputs["v_cache"], dtype=np.float32)
    ln_w = np.asarray(inputs["ln_w"], dtype=np.float32)
    ln_b = np.asarray(inputs["ln_b"], dtype=np.float32)
    attn_w = np.asarray(inputs["attn_w"], dtype=np.float32)
    attn_b = np.asarray(inputs["attn_b"], dtype=np.float32)
    proj_w = np.asarray(inputs["proj_w"], dtype=np.float32)
    proj_b = np.asarray(inputs["proj_b"], dtype=np.float32)
    bf = ml_dtypes.bfloat16

    scale = np.float32(1.0 / np.sqrt(D))
    wln = attn_w * ln_w[:, None]                  # [H, 6144]
    cfull = ln_b @ attn_w + attn_b                # [6144]
    mask = (np.arange(128)[:, None] <= np.arange(512)[None, :]).astype(bf)
    onescol = np.ones((128, 1), dtype=bf)
    ident = np.eye(128, dtype=bf)

    in_maps = []
    for c in range(8):
        b, g = c // 2, c % 2
        hsl = slice(g * NHL, (g + 1) * NHL)
        qsl = slice(g * 1024, (g + 1) * 1024)
        ksl = slice(2048 + g * 1024, 2048 + (g + 1) * 1024)
        vsl = slice(4096 + g * 1024, 4096 + (g + 1) * 1024)
        wq = wln[:, qsl] * scale                  # [2048, 1024]
        wk = wln[:, ksl]
        wv = wln[:, vsl]
        # per-hp stationary layout [4, 128, 16, 768]
        wqkv = np.empty((4, 128, 16, 768), dtype=bf)
        for hp in range(4):
            cols = np.concatenate([
                wq[:, hp * 256:(hp + 1) * 256],
                wk[:, hp * 256:(hp + 1) * 256],
                wv[:, hp * 256:(hp + 1) * 256]], axis=1)   # [2048, 768]
            wqkv[hp] = cols.reshape(16, 128, 768).transpose(1, 0, 2).astype(bf)
        qb = cfull[qsl] * scale
        kb = cfull[ksl]
        vb = cfull[vsl]
        qkbias = np.zeros((128, 16), dtype=np.float32)
        for hp in range(4):
            for jh in range(2):
                qkbias[:, hp * 4 + jh] = qb[hp * 256 + jh * 128:
                                            hp * 256 + (jh + 1) * 128]
                qkbias[:, hp * 4 + 2 + jh] = kb[hp * 256 + jh * 128:
                                                hp * 256 + (jh + 1) * 128]
        kcT = np.ascontiguousarray(
            k_cache[b, :C, hsl, :].transpose(2, 1, 0)).astype(bf)  # [128,8,512]
        vcp = np.ascontiguousarray(
            v_cache[b, :C, hsl, :].reshape(4, 128, NHL, D)
            .transpose(1, 0, 2, 3)).astype(bf)                     # [128,4,8,128]
        pw = np.ascontiguousarray(
            proj_w[qsl, :].reshape(NHL, 128, H).transpose(1, 0, 2)).astype(bf)
        in_maps.append({
            "x": np.ascontiguousarray(hidden[b * L:(b + 1) * L]),
            "wqkv": wqkv,
            "qkbias": qkbias,
            "vbias": vb.reshape(1, 1024).astype(bf),
            "kcT": kcT, "vcp": vcp, "pw": pw,
            "pb": (proj_b if g == 0 else np.zeros_like(proj_b)
                   ).reshape(1, H).astype(bf),
            "mask": mask, "onescol": onescol, "ident": ident,
        })
    return in_maps


def kernel(**inputs) -> np.ndarray:
    if "nc" not in _CACHE:
        _CACHE["nc"] = _build()
    nc = _CACHE["nc"]
    in_maps = _host_prep(inputs)
    res = run_bass_kernel_spmd(nc, in_maps, list(range(8)))
    out = np.concatenate([res.results[c]["out"] for c in range(8)], axis=0)
    return out.astype(np.float32)


# revision 4
# speedup vs baseline: 1.0523x; 1.0523x over previous
"""Trainium2 Bass kernel v2 for CausalSelfAttentionLayer (B=4 L=1024 C=512).

Sharding over 8 NeuronCores: core c -> sequence b=c//2, head-group g=c%2
(8 of 16 heads).  All GEMMs run in bf16 (stationary+moving) with f32 PSUM
accumulate.  Per core:
  A: LN (bn_stats DVE, apply on ACT -> bf16), x-transpose via DMA XBAR
     (dma_start_transpose), weights prefetched per head-pair (double buffer).
  B: per head-pair hp: QKV matmuls (bias folded into the PSUM->SBUF copy on
     ACT/DVE), attention with transposed probabilities, causal boundary
     narrowing (masked-out column ranges skipped), exp on ACT -> bf16,
     row-sums via ones-matmul accumulated in PSUM, normalize on DVE.
  C: c_proj partials (bf16) in token-pair order (j, j+4) with 4 chunked
     pair-ReduceScatters so all but the last overlap compute; out write per
     chunk (bf16->f32 DMA convert).
"""
import sys

for _p in ("/opt/trn_rl_repo", "/root/.axon_site/_ro/trn_rl_repo"):
    if _p not in sys.path:
        sys.path.append(_p)

from contextlib import ExitStack

import numpy as np

import concourse.bass as bass
import concourse.mybir as mybir
import concourse.tile as tile
from concourse import bacc
from concourse.bass_utils import run_bass_kernel_spmd

B, L, C, H, D = 4, 1024, 512, 2048, 128
NHL = 8          # heads per core
NPAIR = 4        # head pairs per core
T = L            # tokens per core (one sequence)
EPS = 1e-5
f32 = mybir.dt.float32
bf16 = mybir.dt.bfloat16

_CACHE = {}


def _build(collective=True, nbody=1, nrs=1):
    nc = bacc.Bacc("TRN2", target_bir_lowering=False, debug=False, num_devices=8)

    x_t = nc.dram_tensor("x", [T, H], f32, kind="ExternalInput")
    # per head-pair stationary weights [128, 16 cgroups, 768 cols]
    # col order: q jh0, q jh1, k jh0, k jh1, v (256)
    w_t = nc.dram_tensor("wqkv", [NPAIR, 128, 16, 768], bf16, kind="ExternalInput")
    qkb_t = nc.dram_tensor("qkbias", [128, 16], f32, kind="ExternalInput")
    vb_t = nc.dram_tensor("vbias", [1, 1024], bf16, kind="ExternalInput")
    kc_t = nc.dram_tensor("kcT", [128, NHL, C], bf16, kind="ExternalInput")
    vc_t = nc.dram_tensor("vcp", [128, 4, NHL, D], bf16, kind="ExternalInput")
    pw_t = nc.dram_tensor("pw", [128, NHL, H], bf16, kind="ExternalInput")
    pb_t = nc.dram_tensor("pb", [1, H], bf16, kind="ExternalInput")
    mask_t = nc.dram_tensor("mask", [128, 512], bf16, kind="ExternalInput")
    id_t = nc.dram_tensor("ident", [128, 128], bf16, kind="ExternalInput")
    onc_t = nc.dram_tensor("onescol", [128, 1], bf16, kind="ExternalInput")
    out_t = nc.dram_tensor("out", [T // 2, H], f32, kind="ExternalOutput")

    Exp = mybir.ActivationFunctionType.Exp
    Sqrt = mybir.ActivationFunctionType.Sqrt
    Ident = mybir.ActivationFunctionType.Identity
    mult = mybir.AluOpType.mult

    with tile.TileContext(nc) as tc, ExitStack() as _stk:
        def _pool(name, bufs, **kw):
            return _stk.enter_context(tc.tile_pool(name=name, bufs=bufs, **kw))
        cst = _pool("cst", 1)
        xload = _pool("xload", 2)
        xlnp = _pool("xln", 2)
        st = _pool("st", 8)
        bigx = _pool("bigx", 2)
        wp = _pool("wp", 2)
        pwp = _pool("pwp", 1)
        qkvp = _pool("qkv", 6)
        kvc = _pool("kvc", 4)
        ptp = _pool("ptp", 4)
        rb = _pool("rb", 2)
        otp = _pool("otp", 1)
        po = _pool("po", 2)
        psS = _pool("psS", 8, space="PSUM")
        dram = _pool("dram", 1, space="DRAM")

        # ---- constants ----
        mask = cst.tile([128, 512], bf16)
        nc.gpsimd.dma_start(out=mask, in_=mask_t.ap())
        ident = cst.tile([128, 128], bf16)
        nc.gpsimd.dma_start(out=ident, in_=id_t.ap())
        onescol = cst.tile([128, 1], bf16)
        nc.gpsimd.dma_start(out=onescol, in_=onc_t.ap())
        qkbias = cst.tile([128, 16], f32)
        nc.gpsimd.dma_start(out=qkbias, in_=qkb_t.ap())
        vbias = cst.tile([128, 1024], bf16)
        nc.gpsimd.dma_start(
            out=vbias,
            in_=bass.AP(tensor=vb_t.ap().tensor, offset=0,
                        ap=[[0, 128], [1, 1024]]))
        pbb = cst.tile([128, H], bf16)
        nc.gpsimd.dma_start(
            out=pbb,
            in_=bass.AP(tensor=pb_t.ap().tensor, offset=0,
                        ap=[[0, 128], [1, H]]))
        epst = cst.tile([128, 1], f32)
        nc.vector.memset(epst, EPS)

        partial = dram.tile([4, 2, 128, H], bf16)
        rsout = dram.tile([4, 128, H], bf16)

        for _rep in range(nbody):
            xTa = bigx.tile([128, 16, T // 2], bf16, tag="xT", name="xTa")
            xTb = bigx.tile([128, 16, T // 2], bf16, tag="xT", name="xTb")

            def ln_chunk(tch):
                """LN one 128-token chunk and DMA-transpose into xT."""
                xh = xload.tile([128, 2048], f32, tag="x", name="xh")
                nc.sync.dma_start(
                    out=xh[:, 0:1024],
                    in_=x_t.ap()[tch * 128:(tch + 1) * 128, 0:1024])
                nc.scalar.dma_start(
                    out=xh[:, 1024:2048],
                    in_=x_t.ap()[tch * 128:(tch + 1) * 128, 1024:2048])
                stats = st.tile([128, 4, 6], f32, tag="stats")
                for sg in range(4):
                    nc.vector.bn_stats(
                        out=stats[:, sg, :],
                        in_=xh[:, sg * 512:(sg + 1) * 512])
                mv = st.tile([128, 2], f32, tag="mv")
                nc.vector.bn_aggr(out=mv, in_=stats)
                stdt = st.tile([128, 1], f32, tag="std")
                nc.scalar.activation(stdt, mv[:, 1:2], Sqrt, bias=epst[:, 0:1])
                rstd = st.tile([128, 1], f32, tag="rstd")
                nc.vector.reciprocal(rstd, stdt)
                m2p = st.tile([128, 1], f32, tag="m2p")
                nc.vector.tensor_scalar(
                    out=m2p, in0=mv[:, 0:1],
                    scalar1=rstd[:, 0:1], scalar2=-1.0, op0=mult, op1=mult)
                xln = xlnp.tile([128, 2048], bf16, tag="xln", name="xln")
                nc.scalar.activation(
                    xln[:], xh[:], Ident,
                    bias=m2p[:, 0:1], scale=rstd[:, 0:1])
                xTh = xTa if tch < 4 else xTb
                tc4 = (tch % 4) * 128
                for g4 in range(4):
                    tp = psS.tile([128, 512], f32, tag="psS", name="tp")
                    for i4 in range(4):
                        nc.tensor.transpose(
                            tp[:, i4 * 64:(i4 + 1) * 64].bitcast(bf16),
                            xln[:, (g4 * 4 + i4) * 128:(g4 * 4 + i4 + 1) * 128],
                            ident)
                    nc.vector.tensor_copy(
                        out=xTh[:, g4 * 4:(g4 + 1) * 4, tc4:tc4 + 128],
                        in_=tp[:, 0:256].bitcast(bf16))

            def load_w(hp):
                w = wp.tile([128, 16, 768], bf16, tag="w", name=f"w{hp}")
                nc.scalar.dma_start(out=w, in_=w_t.ap()[hp])
                return w

            def load_cache(hp):
                kcT = kvc.tile([128, 2, C], bf16, tag="kv", name="kcT")
                nc.scalar.dma_start(
                    out=kcT, in_=kc_t.ap()[:, hp * 2:hp * 2 + 2, :])
                vcp = kvc.tile([128, 4, 2, 128], bf16, tag="kv", name="vcp")
                nc.scalar.dma_start(
                    out=vcp, in_=vc_t.ap()[:, :, hp * 2:hp * 2 + 2, :])
                return kcT, vcp

            def emit_qk(dst, w, hp, qk, jh, tt):
                """One [128 feat, 512 tok] slab of qT or kT."""
                xTh = xTa if tt == 0 else xTb
                pq = psS.tile([128, 512], f32, tag="psS", name="pq")
                for cg in range(16):
                    nc.tensor.matmul(
                        pq[:],
                        w[:, cg, qk * 256 + jh * 128:qk * 256 + (jh + 1) * 128],
                        xTh[:, cg, :],
                        start=(cg == 0), stop=(cg == 15))
                nc.scalar.activation(
                    dst[:, jh, tt * 512:(tt + 1) * 512], pq[:], Ident,
                    bias=qkbias[:, hp * 4 + qk * 2 + jh:hp * 4 + qk * 2 + jh + 1])

            def emit_v(vtok, w, hp, tch):
                xTh = xTa if tch < 4 else xTb
                pv = psS.tile([128, 512], f32, tag="psS", name="pv")
                for cg in range(16):
                    nc.tensor.matmul(
                        pv[:, 0:256],
                        xTh[:, cg, (tch % 4) * 128:(tch % 4 + 1) * 128],
                        w[:, cg, 512:768],
                        start=(cg == 0), stop=(cg == 15))
                nc.vector.tensor_add(
                    vtok[:, tch, :], pv[:, 0:256],
                    vbias[:, hp * 256:(hp + 1) * 256])

            def emit_attn(outT, qT, kT, vtok, kcT, vcp, hp, hl, qt):
                h = hp * 2 + hl
                # chunk list: (kind, idx, mask_start or None, col offset)
                chunks = [("c", pc, None, 0) for pc in range(4)]
                for kf in range(8):
                    dlt = qt * 512 - kf * 128
                    if dlt <= -512:
                        continue
                    ms = dlt + 384 if dlt < 127 else None
                    off = max(0, -dlt)
                    chunks.append(("f", kf, ms, off))

                po_ps = psS.tile([128, 512], f32, tag="psS", name="po_ps")
                sm_ps = psS.tile([128, 512], f32, tag="psS", name="sm_ps")

                pending = []

                def stage1(kind, idx, ms, off):
                    w = 512 - off
                    sc_ps = psS.tile([128, 512], f32, tag="psS", name="sc_ps")
                    lhs = (kcT[:, hl, idx * 128:(idx + 1) * 128] if kind == "c"
                           else kT[:, hl, idx * 128:(idx + 1) * 128])
                    nc.tensor.matmul(
                        sc_ps[:, 0:w], lhs,
                        qT[:, hl, qt * 512 + off:(qt + 1) * 512],
                        start=True, stop=True)
                    pt = ptp.tile([128, 512], bf16, tag="pt", name="pt")
                    nc.scalar.activation(pt[:, 0:w], sc_ps[:, 0:w], Exp)
                    if ms is not None:
                        nc.vector.tensor_mul(
                            pt[:, 0:w], pt[:, 0:w],
                            mask[:, 0:ms + 128])
                    return pt

                def stage2(pt, kind, idx, ms, off, first):
                    w = 512 - off
                    vchunk = (vcp[:, idx, hl, :] if kind == "c"
                              else vtok[:, idx, hl * 128:(hl + 1) * 128])
                    nc.tensor.matmul(po_ps[:, off:512], vchunk, pt[:, 0:w],
                                     start=first, stop=False)
                    nc.tensor.matmul(sm_ps[0:1, off:512], onescol, pt[:, 0:w],
                                     start=first, stop=False)

                first = True
                for ci, (kind, idx, ms, off) in enumerate(chunks):
                    pt = stage1(kind, idx, ms, off)
                    pending.append((pt, kind, idx, ms, off))
                    if ci >= 2:
                        args = pending.pop(0)
                        stage2(*args, first)
                        first = False
                for args in pending:
                    stage2(*args, first)
                    first = False

                rc = st.tile([1, 512], bf16, tag="recip", bufs=2)
                with nc.allow_low_precision(reason="softmax denom bf16"):
                    nc.vector.reciprocal(rc, sm_ps[0:1, :])
                rcb = rb.tile([128, 512], bf16, tag="rcb", name="rcb")
                nc.gpsimd.partition_broadcast(rcb[:], rc[:])
                nc.vector.tensor_mul(
                    out=outT[:, h, qt * 512:(qt + 1) * 512],
                    in0=po_ps[:], in1=rcb[:])

            # ---- phase A + hp0 QKV interleave ----
            w0 = load_w(0)
            kcT0, vcp0 = load_cache(0)
            outT = otp.tile([128, NHL, T], bf16, tag="outT")
            qT0 = qkvp.tile([128, 2, T], bf16, tag="qkv", name="qT0")
            kT0 = qkvp.tile([128, 2, T], bf16, tag="qkv", name="kT0")
            vtok0 = qkvp.tile([128, 8, 256], bf16, tag="qkv", name="vtok0")
            for tch in range(4):
                ln_chunk(tch)
            for jh in range(2):
                emit_qk(qT0, w0, 0, 0, jh, 0)
            for jh in range(2):
                emit_qk(kT0, w0, 0, 1, jh, 0)
            for tch in range(4):
                emit_v(vtok0, w0, 0, tch)
            for tch in range(4, 8):
                ln_chunk(tch)
            pw_sb = pwp.tile([128, NHL, H], bf16, tag="pw")
            nc.sync.dma_start(out=pw_sb, in_=pw_t.ap())
            for jh in range(2):
                emit_qk(qT0, w0, 0, 0, jh, 1)
            for jh in range(2):
                emit_qk(kT0, w0, 0, 1, jh, 1)
            for tch in range(4, 8):
                emit_v(vtok0, w0, 0, tch)

            # ---- phase B ----
            cur_w, cur_kc, cur_vc = w0, kcT0, vcp0
            qT, kT, vtok = qT0, kT0, vtok0
            for hp in range(NPAIR):
                if hp > 0:
                    qT = qkvp.tile([128, 2, T], bf16, tag="qkv", name="qT")
                    kT = qkvp.tile([128, 2, T], bf16, tag="qkv", name="kT")
                    vtok = qkvp.tile([128, 8, 256], bf16, tag="qkv",
                                     name="vtok")
                    for jh in range(2):
                        for tt in range(2):
                            emit_qk(qT, cur_w, hp, 0, jh, tt)
                    for jh in range(2):
                        for tt in range(2):
                            emit_qk(kT, cur_w, hp, 1, jh, tt)
                    for tch in range(8):
                        emit_v(vtok, cur_w, hp, tch)
                nxt = None
                if hp + 1 < NPAIR:
                    nxt = (load_w(hp + 1),) + load_cache(hp + 1)
                for qt in range(2):
                    for hl in range(2):
                        emit_attn(outT, qT, kT, vtok, cur_kc, cur_vc,
                                  hp, hl, qt)
                if nxt is not None:
                    cur_w, cur_kc, cur_vc = nxt

            # ---- phase C: c_proj partials + chunked pair-ReduceScatter ----
            for j in range(4):
                for gb in range(2):
                    tch = gb * 4 + j
                    for ht in range(4):
                        pp = psS.tile([128, 512], f32, tag="psS", name="pp")
                        for hh in range(NHL):
                            nc.tensor.matmul(
                                pp[:],
                                outT[:, hh, tch * 128:(tch + 1) * 128],
                                pw_sb[:, hh, ht * 512:(ht + 1) * 512],
                                start=(hh == 0), stop=(hh == NHL - 1))
                        ev = po.tile([128, 512], bf16, tag="po", name="ev")
                        nc.vector.tensor_add(ev[:], pp[:],
                                             pbb[:, ht * 512:(ht + 1) * 512])
                        nc.sync.dma_start(
                            out=partial[j, gb, :, ht * 512:(ht + 1) * 512],
                            in_=ev[:])
                if collective:
                    for _ in range(nrs):
                        nc.gpsimd.collective_compute(
                            "ReduceScatter",
                            mybir.AluOpType.add,
                            replica_groups=[[0, 1], [2, 3], [4, 5], [6, 7]],
                            ins=[partial[j]],
                            outs=[rsout[j]],
                        )
                    nc.gpsimd.dma_start(
                        out=out_t.ap()[j * 128:(j + 1) * 128, :],
                        in_=rsout[j])
                else:
                    nc.gpsimd.dma_start(
                        out=out_t.ap()[j * 128:(j + 1) * 128, :],
                        in_=partial[j, 0, :, :])

    nc.compile()
    return nc


def _host_prep(inputs):
    import ml_dtypes
    hidden = np.ascontiguousarray(np.asarray(inputs["hidden_states"],
                                             dtype=np.float32))
    k_cache = np.asarray(inputs["k_cache"], dtype=np.float32)
    v_cache = np.asarray(inputs["v_cache"], dtype=np.float32)
    ln_w = np.asarray(inputs["ln_w"], dtype=np.float32)
    ln_b = np.asarray(inputs["ln_b"], dtype=np.float32)
    attn_w = np.asarray(inputs["attn_w"], dtype=np.float32)
    attn_b = np.asarray(inputs["attn_b"], dtype=np.float32)
    proj_w = np.asarray(inputs["proj_w"], dtype=np.float32)
    proj_b = np.asarray(inputs["proj_b"], dtype=np.float32)
    bf = ml_dtypes.bfloat16

    scale = np.float32(1.0 / np.sqrt(D))
    wln = attn_w * ln_w[:, None]                  # [H, 6144]
    cfull = ln_b @ attn_w + attn_b                # [6144]
    mask = (np.arange(128)[:, None] <= np.arange(512)[None, :]).astype(bf)
    onescol = np.ones((128, 1), dtype=bf)
    ident = np.eye(128, dtype=bf)

    in_maps = []
    for c in range(8):
        b, g = c // 2, c % 2
        hsl = slice(g * NHL, (g + 1) * NHL)
        qsl = slice(g * 1024, (g + 1) * 1024)
        ksl = slice(2048 + g * 1024, 2048 + (g + 1) * 1024)
        vsl = slice(4096 + g * 1024, 4096 + (g + 1) * 1024)
        wq = wln[:, qsl] * scale                  # [2048, 1024]
        wk = wln[:, ksl]
        wv = wln[:, vsl]
        # per-hp stationary layout [4, 128, 16, 768]
        wqkv = np.empty((4, 128, 16, 768), dtype=bf)
        for hp in range(4):
            cols = np.concatenate([
                wq[:, hp * 256:(hp + 1) * 256],
                wk[:, hp * 256:(hp + 1) * 256],
                wv[:, hp * 256:(hp + 1) * 256]], axis=1)   # [2048, 768]
            wqkv[hp] = cols.reshape(16, 128, 768).transpose(1, 0, 2).astype(bf)
        qb = cfull[qsl] * scale
        kb = cfull[ksl]
        vb = cfull[vsl]
        qkbias = np.zeros((128, 16), dtype=np.float32)
        for hp in range(4):
            for jh in range(2):
                qkbias[:, hp * 4 + jh] = qb[hp * 256 + jh * 128:
                                            hp * 256 + (jh + 1) * 128]
                qkbias[:, hp * 4 + 2 + jh] = kb[hp * 256 + jh * 128:
                                                hp * 256 + (jh + 1) * 128]
        kcT = np.ascontiguousarray(
            k_cache[b, :C, hsl, :].transpose(2, 1, 0)).astype(bf)  # [128,8,512]
        vcp = np.ascontiguousarray(
            v_cache[b, :C, hsl, :].reshape(4, 128, NHL, D)
            .transpose(1, 0, 2, 3)).astype(bf)                     # [128,4,8,128]
        pw = np.ascontiguousarray(
            proj_w[qsl, :].reshape(NHL, 128, H).transpose(1, 0, 2)).astype(bf)
        in_maps.append({
            "x": np.ascontiguousarray(hidden[b * L:(b + 1) * L]),
            "wqkv": wqkv,
            "qkbias": qkbias,
            "vbias": vb.reshape(1, 1024).astype(bf),
            "kcT": kcT, "vcp": vcp, "pw": pw,
            "pb": (proj_b if g == 0 else np.zeros_like(proj_b)
                   ).reshape(1, H).astype(bf),
            "mask": mask, "onescol": onescol, "ident": ident,
        })
    return in_maps


def kernel(**inputs) -> np.ndarray:
    if "nc" not in _CACHE:
        _CACHE["nc"] = _build()
    nc = _CACHE["nc"]
    in_maps = _host_prep(inputs)
    res = run_bass_kernel_spmd(nc, in_maps, list(range(8)))
    out = np.concatenate([res.results[c]["out"] for c in range(8)], axis=0)
    return out.astype(np.float32)


# revision 7
# speedup vs baseline: 1.7115x; 1.6263x over previous
"""Trainium2 Bass kernel v2 for CausalSelfAttentionLayer (B=4 L=1024 C=512).

Sharding over 8 NeuronCores: core c -> sequence b=c//2, head-group g=c%2
(8 of 16 heads).  All GEMMs run in bf16 (stationary+moving) with f32 PSUM
accumulate.  Per core:
  A: LN (bn_stats DVE, apply on ACT -> bf16), x-transpose via DMA XBAR
     (dma_start_transpose), weights prefetched per head-pair (double buffer).
  B: per head-pair hp: QKV matmuls (bias folded into the PSUM->SBUF copy on
     ACT/DVE), attention with transposed probabilities, causal boundary
     narrowing (masked-out column ranges skipped), exp on ACT -> bf16,
     row-sums via ones-matmul accumulated in PSUM, normalize on DVE.
  C: c_proj partials (bf16) in token-pair order (j, j+4) with 4 chunked
     pair-ReduceScatters so all but the last overlap compute; out write per
     chunk (bf16->f32 DMA convert).
"""
import sys

for _p in ("/opt/trn_rl_repo", "/root/.axon_site/_ro/trn_rl_repo"):
    if _p not in sys.path:
        sys.path.append(_p)

from contextlib import ExitStack

import numpy as np

import concourse.bass as bass
import concourse.mybir as mybir
import concourse.tile as tile
from concourse import bacc
from concourse.bass_utils import run_bass_kernel_spmd

B, L, C, H, D = 4, 1024, 512, 2048, 128
NHL = 8          # heads per core
NPAIR = 4        # head pairs per core
T = L            # tokens per core (one sequence)
EPS = 1e-5
f32 = mybir.dt.float32
bf16 = mybir.dt.bfloat16

_CACHE = {}


def _build(collective=True, nbody=1, nrs=1):
    nc = bacc.Bacc("TRN2", target_bir_lowering=False, debug=False, num_devices=8)

    x_t = nc.dram_tensor("x", [T, H], f32, kind="ExternalInput")
    # per head-pair stationary weights [128, 16 cgroups, 768 cols]
    # col order: q jh0, q jh1, k jh0, k jh1, v (256)
    w_t = nc.dram_tensor("wqkv", [NPAIR, 128, 16, 768], bf16, kind="ExternalInput")
    qkb_t = nc.dram_tensor("qkbias", [128, 16], f32, kind="ExternalInput")
    vb_t = nc.dram_tensor("vbias", [1, 1024], bf16, kind="ExternalInput")
    kc_t = nc.dram_tensor("kcT", [128, NHL, C], bf16, kind="ExternalInput")
    vc_t = nc.dram_tensor("vcp", [128, 4, NHL, D], bf16, kind="ExternalInput")
    pw_t = nc.dram_tensor("pw", [128, NHL, H], bf16, kind="ExternalInput")
    pb_t = nc.dram_tensor("pb", [1, H], bf16, kind="ExternalInput")
    mask_t = nc.dram_tensor("mask", [128, 512], bf16, kind="ExternalInput")
    id_t = nc.dram_tensor("ident", [128, 128], bf16, kind="ExternalInput")
    onc_t = nc.dram_tensor("onescol", [128, 1], bf16, kind="ExternalInput")
    out_t = nc.dram_tensor("out", [T // 2, H], f32, kind="ExternalOutput")

    Exp = mybir.ActivationFunctionType.Exp
    Sqrt = mybir.ActivationFunctionType.Sqrt
    Ident = mybir.ActivationFunctionType.Identity
    mult = mybir.AluOpType.mult

    with tile.TileContext(nc) as tc, ExitStack() as _stk:
        def _pool(name, bufs, **kw):
            return _stk.enter_context(tc.tile_pool(name=name, bufs=bufs, **kw))
        cst = _pool("cst", 1)
        xload = _pool("xload", 2)
        xlnp = _pool("xln", 2)
        st = _pool("st", 8)
        bigx = _pool("bigx", 2)
        wp = _pool("wp", 2)
        pwp = _pool("pwp", 1)
        qkvp = _pool("qkv", 6)
        kvc = _pool("kvc", 4)
        ptp = _pool("ptp", 4)
        rb = _pool("rb", 2)
        otp = _pool("otp", 1)
        po = _pool("po", 2)
        psS = _pool("psS", 8, space="PSUM")
        dram = _pool("dram", 1, space="DRAM")

        # ---- constants ----
        mask = cst.tile([128, 512], bf16)
        nc.gpsimd.dma_start(out=mask, in_=mask_t.ap())
        ident = cst.tile([128, 128], bf16)
        nc.gpsimd.dma_start(out=ident, in_=id_t.ap())
        onescol = cst.tile([128, 1], bf16)
        nc.gpsimd.dma_start(out=onescol, in_=onc_t.ap())
        qkbias = cst.tile([128, 16], f32)
        nc.gpsimd.dma_start(out=qkbias, in_=qkb_t.ap())
        vbias = cst.tile([128, 1024], bf16)
        nc.gpsimd.dma_start(
            out=vbias,
            in_=bass.AP(tensor=vb_t.ap().tensor, offset=0,
                        ap=[[0, 128], [1, 1024]]))
        pbb = cst.tile([128, H], bf16)
        nc.gpsimd.dma_start(
            out=pbb,
            in_=bass.AP(tensor=pb_t.ap().tensor, offset=0,
                        ap=[[0, 128], [1, H]]))
        epst = cst.tile([128, 1], f32)
        nc.vector.memset(epst, EPS)

        partial = dram.tile([4, 2, 128, H], bf16)
        rsout = dram.tile([4, 128, H], bf16)

        for _rep in range(nbody):
            xTa = bigx.tile([128, 16, T // 2], bf16, tag="xT", name="xTa")
            xTb = bigx.tile([128, 16, T // 2], bf16, tag="xT", name="xTb")

            def ln_chunk(tch):
                """LN one 128-token chunk and DMA-transpose into xT."""
                xh = xload.tile([128, 2048], f32, tag="x", name="xh")
                nc.sync.dma_start(
                    out=xh[:, 0:1024],
                    in_=x_t.ap()[tch * 128:(tch + 1) * 128, 0:1024])
                nc.scalar.dma_start(
                    out=xh[:, 1024:2048],
                    in_=x_t.ap()[tch * 128:(tch + 1) * 128, 1024:2048])
                stats = st.tile([128, 4, 6], f32, tag="stats")
                for sg in range(4):
                    nc.vector.bn_stats(
                        out=stats[:, sg, :],
                        in_=xh[:, sg * 512:(sg + 1) * 512])
                mv = st.tile([128, 2], f32, tag="mv")
                nc.vector.bn_aggr(out=mv, in_=stats)
                stdt = st.tile([128, 1], f32, tag="std")
                nc.scalar.activation(stdt, mv[:, 1:2], Sqrt, bias=epst[:, 0:1])
                rstd = st.tile([128, 1], f32, tag="rstd")
                nc.vector.reciprocal(rstd, stdt)
                m2p = st.tile([128, 1], f32, tag="m2p")
                nc.vector.tensor_scalar(
                    out=m2p, in0=mv[:, 0:1],
                    scalar1=rstd[:, 0:1], scalar2=-1.0, op0=mult, op1=mult)
                xln = xlnp.tile([128, 2048], bf16, tag="xln", name="xln")
                nc.scalar.activation(
                    xln[:], xh[:], Ident,
                    bias=m2p[:, 0:1], scale=rstd[:, 0:1])
                xTh = xTa if tch < 4 else xTb
                tc4 = (tch % 4) * 128
                for g4 in range(4):
                    tp = psS.tile([128, 512], f32, tag="psS", name="tp")
                    for i4 in range(4):
                        nc.tensor.transpose(
                            tp[:, i4 * 64:(i4 + 1) * 64].bitcast(bf16),
                            xln[:, (g4 * 4 + i4) * 128:(g4 * 4 + i4 + 1) * 128],
                            ident)
                    nc.vector.tensor_copy(
                        out=xTh[:, g4 * 4:(g4 + 1) * 4, tc4:tc4 + 128],
                        in_=tp[:, 0:256].bitcast(bf16))

            def load_w(hp):
                w = wp.tile([128, 16, 768], bf16, tag="w", name=f"w{hp}")
                nc.scalar.dma_start(out=w, in_=w_t.ap()[hp])
                return w

            def load_cache(hp):
                kcT = kvc.tile([128, 2, C], bf16, tag="kv", name="kcT")
                nc.scalar.dma_start(
                    out=kcT, in_=kc_t.ap()[:, hp * 2:hp * 2 + 2, :])
                vcp = kvc.tile([128, 4, 2, 128], bf16, tag="kv", name="vcp")
                nc.scalar.dma_start(
                    out=vcp, in_=vc_t.ap()[:, :, hp * 2:hp * 2 + 2, :])
                return kcT, vcp

            def emit_qk(dst, w, hp, qk, jh, tt):
                """One [128 feat, 512 tok] slab of qT or kT."""
                xTh = xTa if tt == 0 else xTb
                pq = psS.tile([128, 512], f32, tag="psS", name="pq")
                for cg in range(16):
                    nc.tensor.matmul(
                        pq[:],
                        w[:, cg, qk * 256 + jh * 128:qk * 256 + (jh + 1) * 128],
                        xTh[:, cg, :],
                        start=(cg == 0), stop=(cg == 15))
                nc.scalar.activation(
                    dst[:, jh, tt * 512:(tt + 1) * 512], pq[:], Ident,
                    bias=qkbias[:, hp * 4 + qk * 2 + jh:hp * 4 + qk * 2 + jh + 1])

            def emit_v(vtok, w, hp, tch):
                xTh = xTa if tch < 4 else xTb
                pv = psS.tile([128, 512], f32, tag="psS", name="pv")
                for cg in range(16):
                    nc.tensor.matmul(
                        pv[:, 0:256],
                        xTh[:, cg, (tch % 4) * 128:(tch % 4 + 1) * 128],
                        w[:, cg, 512:768],
                        start=(cg == 0), stop=(cg == 15))
                nc.vector.tensor_add(
                    vtok[:, tch, :], pv[:, 0:256],
                    vbias[:, hp * 256:(hp + 1) * 256])

            def emit_attn(outT, qT, kT, vtok, kcT, vcp, hp, hl, qt):
                h = hp * 2 + hl
                # chunk list: (kind, idx, mask_start or None, col offset)
                chunks = [("c", pc, None, 0) for pc in range(4)]
                for kf in range(8):
                    dlt = qt * 512 - kf * 128
                    if dlt <= -512:
                        continue
                    ms = dlt + 384 if dlt < 127 else None
                    off = max(0, -dlt)
                    chunks.append(("f", kf, ms, off))

                po_ps = psS.tile([128, 512], f32, tag="psS", name="po_ps")
                sm_ps = psS.tile([128, 512], f32, tag="psS", name="sm_ps")

                pending = []

                def stage1(kind, idx, ms, off):
                    w = 512 - off
                    sc_ps = psS.tile([128, 512], f32, tag="psS", name="sc_ps")
                    lhs = (kcT[:, hl, idx * 128:(idx + 1) * 128] if kind == "c"
                           else kT[:, hl, idx * 128:(idx + 1) * 128])
                    nc.tensor.matmul(
                        sc_ps[:, 0:w], lhs,
                        qT[:, hl, qt * 512 + off:(qt + 1) * 512],
                        start=True, stop=True)
                    pt = ptp.tile([128, 512], bf16, tag="pt", name="pt")
                    nc.scalar.activation(pt[:, 0:w], sc_ps[:, 0:w], Exp)
                    if ms is not None:
                        nc.vector.tensor_mul(
                            pt[:, 0:w], pt[:, 0:w],
                            mask[:, 0:ms + 128])
                    return pt

                def stage2(pt, kind, idx, ms, off, first):
                    w = 512 - off
                    vchunk = (vcp[:, idx, hl, :] if kind == "c"
                              else vtok[:, idx, hl * 128:(hl + 1) * 128])
                    nc.tensor.matmul(po_ps[:, off:512], vchunk, pt[:, 0:w],
                                     start=first, stop=False)
                    nc.tensor.matmul(sm_ps[0:1, off:512], onescol, pt[:, 0:w],
                                     start=first, stop=False)

                first = True
                for ci, (kind, idx, ms, off) in enumerate(chunks):
                    pt = stage1(kind, idx, ms, off)
                    pending.append((pt, kind, idx, ms, off))
                    if ci >= 2:
                        args = pending.pop(0)
                        stage2(*args, first)
                        first = False
                for args in pending:
                    stage2(*args, first)
                    first = False

                rc = st.tile([1, 512], bf16, tag="recip", bufs=2)
                with nc.allow_low_precision(reason="softmax denom bf16"):
                    nc.vector.reciprocal(rc, sm_ps[0:1, :])
                rcb = rb.tile([128, 512], bf16, tag="rcb", name="rcb")
                nc.gpsimd.partition_broadcast(rcb[:], rc[:])
                nc.vector.tensor_mul(
                    out=outT[:, h, qt * 512:(qt + 1) * 512],
                    in0=po_ps[:], in1=rcb[:])

            # ---- phase A + hp0 QKV interleave ----
            outT = otp.tile([128, NHL, T], bf16, tag="outT")
            qT0 = qkvp.tile([128, 2, T], bf16, tag="qkv", name="qT0")
            kT0 = qkvp.tile([128, 2, T], bf16, tag="qkv", name="kT0")
            vtok0 = qkvp.tile([128, 8, 256], bf16, tag="qkv", name="vtok0")
            for tch in range(4):
                ln_chunk(tch)
            w0 = load_w(0)
            for jh in range(2):
                emit_qk(qT0, w0, 0, 0, jh, 0)
            for jh in range(2):
                emit_qk(kT0, w0, 0, 1, jh, 0)
            for tch in range(4):
                emit_v(vtok0, w0, 0, tch)
            for tch in range(4, 8):
                ln_chunk(tch)
            kcT0, vcp0 = load_cache(0)
            pw_sb = pwp.tile([128, NHL, H], bf16, tag="pw")
            nc.sync.dma_start(out=pw_sb, in_=pw_t.ap())
            for jh in range(2):
                emit_qk(qT0, w0, 0, 0, jh, 1)
            for jh in range(2):
                emit_qk(kT0, w0, 0, 1, jh, 1)
            for tch in range(4, 8):
                emit_v(vtok0, w0, 0, tch)

            # ---- phase B ----
            cur_w, cur_kc, cur_vc = w0, kcT0, vcp0
            qT, kT, vtok = qT0, kT0, vtok0
            for hp in range(NPAIR):
                if hp > 0:
                    qT = qkvp.tile([128, 2, T], bf16, tag="qkv", name="qT")
                    kT = qkvp.tile([128, 2, T], bf16, tag="qkv", name="kT")
                    vtok = qkvp.tile([128, 8, 256], bf16, tag="qkv",
                                     name="vtok")
                    for jh in range(2):
                        for tt in range(2):
                            emit_qk(qT, cur_w, hp, 0, jh, tt)
                    for jh in range(2):
                        for tt in range(2):
                            emit_qk(kT, cur_w, hp, 1, jh, tt)
                    for tch in range(8):
                        emit_v(vtok, cur_w, hp, tch)
                nxt = None
                if hp + 1 < NPAIR:
                    nxt = (load_w(hp + 1),) + load_cache(hp + 1)
                for qt in range(2):
                    for hl in range(2):
                        emit_attn(outT, qT, kT, vtok, cur_kc, cur_vc,
                                  hp, hl, qt)
                if nxt is not None:
                    cur_w, cur_kc, cur_vc = nxt

            # ---- phase C: c_proj partials + chunked pair-ReduceScatter ----
            for j in range(4):
                for gb in range(2):
                    tch = gb * 4 + j
                    for ht in range(4):
                        pp = psS.tile([128, 512], f32, tag="psS", name="pp")
                        for hh in range(NHL):
                            nc.tensor.matmul(
                                pp[:],
                                outT[:, hh, tch * 128:(tch + 1) * 128],
                                pw_sb[:, hh, ht * 512:(ht + 1) * 512],
                                start=(hh == 0), stop=(hh == NHL - 1))
                        ev = po.tile([128, 512], bf16, tag="po", name="ev")
                        nc.vector.tensor_add(ev[:], pp[:],
                                             pbb[:, ht * 512:(ht + 1) * 512])
                        nc.sync.dma_start(
                            out=partial[j, gb, :, ht * 512:(ht + 1) * 512],
                            in_=ev[:])
                if collective:
                    for _ in range(nrs):
                        nc.gpsimd.collective_compute(
                            "ReduceScatter",
                            mybir.AluOpType.add,
                            replica_groups=[[0, 1], [2, 3], [4, 5], [6, 7]],
                            ins=[partial[j]],
                            outs=[rsout[j]],
                        )
                    nc.gpsimd.dma_start(
                        out=out_t.ap()[j * 128:(j + 1) * 128, :],
                        in_=rsout[j])
                else:
                    nc.gpsimd.dma_start(
                        out=out_t.ap()[j * 128:(j + 1) * 128, :],
                        in_=partial[j, 0, :, :])

    nc.compile()
    return nc


def _host_prep(inputs):
    import ml_dtypes
    hidden = np.ascontiguousarray(np.asarray(inputs["hidden_states"],
                                             dtype=np.float32))
    k_cache = np.asarray(inputs["k_cache"], dtype=np.float32)
    v_cache = np.asarray(inputs["v_cache"], dtype=np.float32)
    ln_w = np.asarray(inputs["ln_w"], dtype=np.float32)
    ln_b = np.asarray(inputs["ln_b"], dtype=np.float32)
    attn_w = np.asarray(inputs["attn_w"], dtype=np.float32)
    attn_b = np.asarray(inputs["attn_b"], dtype=np.float32)
    proj_w = np.asarray(inputs["proj_w"], dtype=np.float32)
    proj_b = np.asarray(inputs["proj_b"], dtype=np.float32)
    bf = ml_dtypes.bfloat16

    scale = np.float32(1.0 / np.sqrt(D))
    wln = attn_w * ln_w[:, None]                  # [H, 6144]
    cfull = ln_b @ attn_w + attn_b                # [6144]
    mask = (np.arange(128)[:, None] <= np.arange(512)[None, :]).astype(bf)
    onescol = np.ones((128, 1), dtype=bf)
    ident = np.eye(128, dtype=bf)

    in_maps = []
    for c in range(8):
        b, g = c // 2, c % 2
        hsl = slice(g * NHL, (g + 1) * NHL)
        qsl = slice(g * 1024, (g + 1) * 1024)
        ksl = slice(2048 + g * 1024, 2048 + (g + 1) * 1024)
        vsl = slice(4096 + g * 1024, 4096 + (g + 1) * 1024)
        wq = wln[:, qsl] * scale                  # [2048, 1024]
        wk = wln[:, ksl]
        wv = wln[:, vsl]
        # per-hp stationary layout [4, 128, 16, 768]
        wqkv = np.empty((4, 128, 16, 768), dtype=bf)
        for hp in range(4):
            cols = np.concatenate([
                wq[:, hp * 256:(hp + 1) * 256],
                wk[:, hp * 256:(hp + 1) * 256],
                wv[:, hp * 256:(hp + 1) * 256]], axis=1)   # [2048, 768]
            wqkv[hp] = cols.reshape(16, 128, 768).transpose(1, 0, 2).astype(bf)
        qb = cfull[qsl] * scale
        kb = cfull[ksl]
        vb = cfull[vsl]
        qkbias = np.zeros((128, 16), dtype=np.float32)
        for hp in range(4):
            for jh in range(2):
                qkbias[:, hp * 4 + jh] = qb[hp * 256 + jh * 128:
                                            hp * 256 + (jh + 1) * 128]
                qkbias[:, hp * 4 + 2 + jh] = kb[hp * 256 + jh * 128:
                                                hp * 256 + (jh + 1) * 128]
        kcT = np.ascontiguousarray(
            k_cache[b, :C, hsl, :].transpose(2, 1, 0)).astype(bf)  # [128,8,512]
        vcp = np.ascontiguousarray(
            v_cache[b, :C, hsl, :].reshape(4, 128, NHL, D)
            .transpose(1, 0, 2, 3)).astype(bf)                     # [128,4,8,128]
        pw = np.ascontiguousarray(
            proj_w[qsl, :].reshape(NHL, 128, H).transpose(1, 0, 2)).astype(bf)
        in_maps.append({
            "x": np.ascontiguousarray(hidden[b * L:(b + 1) * L]),
            "wqkv": wqkv,
            "qkbias": qkbias,
            "vbias": vb.reshape(1, 1024).astype(bf),
            "kcT": kcT, "vcp": vcp, "pw": pw,
            "pb": (proj_b if g == 0 else np.zeros_like(proj_b)
                   ).reshape(1, H).astype(bf),
            "mask": mask, "onescol": onescol, "ident": ident,
        })
    return in_maps


def kernel(**inputs) -> np.ndarray:
    if "nc" not in _CACHE:
        _CACHE["nc"] = _build()
    nc = _CACHE["nc"]
    in_maps = _host_prep(inputs)
    res = run_bass_kernel_spmd(nc, in_maps, list(range(8)))
    out = np.concatenate([res.results[c]["out"] for c in range(8)], axis=0)
    return out.astype(np.float32)
